# revision 1
# baseline (speedup 1.0000x reference)
"""Causal attention (RMSNorm + QKV proj + causal softmax attention) on 8 TRN2
NeuronCores.

Math (per reference):
    xn   = x / max(||x_row||, 1e-12) * sqrt(D) * gamma
    qkv  = xn @ w_qkv            -> q,k,v heads of dim 64
    q   *= D**-0.5
    out[b,h] = softmax_causal(q k^T) v

Sharding: 16 (batch, head) pairs over 8 cores -> core c gets batch c//4 and
heads {2*(c%4), 2*(c%4)+1}. Each core runs the same single-core Bass program
(SPMD) on its shard; gamma/weight slices are replicated per core.

Per-core kernel layout strategy:
  - row scale s = sqrt(D)/||x_row|| applied to x in natural layout (GpSimd)
  - x transposed to d-major via PE transposes (f32r, bundled 4-wide into PSUM)
  - projections: W (gamma folded, q also 1/32) stationary, xT moving ->
    qT/kT [128=2*64 feat, 4096] and vT; vT re-transposed to v and packed as
    [v | ones] so the attention AV matmul also produces the softmax
    denominator as output row 64.
  - attention in transposed orientation: simT[j,i] = kT.T qT per 128-key x
    512-query block (keys on partitions), exp on ScalarE from PSUM, causal
    zeroing of the 128-wide diagonal strip via affine_select, AV accumulated
    in PSUM over key blocks. Final [65,512] blocks PE-transposed back to
    token-major, divided by the denominator row, DMA'd out.
"""

import os
import numpy as np
from contextlib import ExitStack

import concourse.bass as bass
import concourse.tile as tile
from concourse import bacc, mybir
from concourse.masks import make_identity

F32 = mybir.dt.float32
F32R = mybir.dt.float32r
AF = mybir.ActivationFunctionType
ALU = mybir.AluOpType

B, N, D = 2, 4096, 1024
HEADS, DH = 8, 64
NT = N // 128      # 32 token tiles
NSUP = N // 512    # 8 token superblocks
DC = D // 128      # 8 contraction chunks
VW = 2 * (DH + 1)  # 130: [v_h0 | ones | v_h1 | ones] per key tile


def build_program():
    nc = bacc.Bacc("TRN2", target_bir_lowering=False, debug=False)

    x = nc.dram_tensor("x", [N, D], F32, kind="ExternalInput").ap()
    wq = nc.dram_tensor("wq", [D, 128], F32, kind="ExternalInput").ap()
    wk = nc.dram_tensor("wk", [D, 128], F32, kind="ExternalInput").ap()
    wv = nc.dram_tensor("wv", [D, 128], F32, kind="ExternalInput").ap()
    gamma = nc.dram_tensor("gamma", [D], F32, kind="ExternalInput").ap()
    out = nc.dram_tensor("out", [2, N, DH], F32, kind="ExternalOutput").ap()

    with tile.TileContext(nc) as tc, ExitStack() as ctx:
        consts = ctx.enter_context(tc.tile_pool(name="consts", bufs=1))
        wpool = ctx.enter_context(tc.tile_pool(name="wpool", bufs=1))
        _ph0 = os.environ.get("KERN_PHASE", "full")
        xpool = ctx.enter_context(
            tc.tile_pool(name="xpool", bufs=4 if _ph0 == "att_norowtile" else 5))
        scpool = ctx.enter_context(
            tc.tile_pool(name="scpool", bufs=1 if _ph0 == "att_norowtile" else 2))
        spool = ctx.enter_context(tc.tile_pool(name="spool", bufs=10))
        xtpool = ctx.enter_context(
            tc.tile_pool(name="xtpool", bufs=1 if _ph0 == "att_norowtile" else 2))
        resid = ctx.enter_context(tc.tile_pool(name="resid", bufs=1))
        vtpool = ctx.enter_context(
            tc.tile_pool(name="vtpool", bufs=1 if _ph0 == "att_norowtile" else 2))
        expool = ctx.enter_context(
            tc.tile_pool(name="expool", bufs=2 if _ph0 == "att_norowtile" else (8 if os.environ.get("KERN_TUNE", "") == "1" else 6)))
        opool = ctx.enter_context(tc.tile_pool(name="opool", bufs=2))
        finpool = ctx.enter_context(tc.tile_pool(name="finpool", bufs=4))
        _tune = os.environ.get("KERN_TUNE", "") == "1"
        psA = ctx.enter_context(
            tc.tile_pool(name="psA", bufs=5 if _tune else 4, space="PSUM"))
        psB = ctx.enter_context(tc.tile_pool(name="psB", bufs=2, space="PSUM"))
        psC = ctx.enter_context(
            tc.tile_pool(name="psC", bufs=1 if _tune else 2, space="PSUM"))

        # ---- Phase 0: constants + weights -------------------------------
        ident = consts.tile([128, 128], F32)
        make_identity(nc, ident[:])

        w_raw = {}
        w_sb = {}
        for name, w in (("q", wq), ("k", wk), ("v", wv)):
            t = wpool.tile([128, DC, 128], F32, tag=f"wr{name}")
            for c in range(DC):
                nc.sync.dma_start(t[:, c, :], w[c * 128:(c + 1) * 128, :])
            w_raw[name] = t
            w_sb[name] = wpool.tile([128, DC, 128], F32R, tag=f"w{name}", name=f"w_{name}")

        g_sb = wpool.tile([128, DC], F32, tag="g")
        nc.sync.dma_start(g_sb[:], gamma.rearrange("(c p) -> p c", p=128))
        gq_sb = wpool.tile([128, DC], F32, tag="gq")
        nc.scalar.mul(gq_sb[:], g_sb[:], float(D) ** -0.5)

        for name, gt in (("q", gq_sb), ("k", g_sb), ("v", g_sb)):
            for c in range(DC):
                nc.vector.tensor_scalar_mul(
                    w_sb[name][:, c, :], w_raw[name][:, c, :], gt[:, c:c + 1])

        # qT/kT: [128 feat (2 heads x 64), N] resident
        qT = resid.tile([128, N], F32R, tag="qT")
        kT = resid.tile([128, N], F32R, tag="kT")
        # v_ext: per key tile jj: cols [jj*130 .. ) = [v_h0(64) | 1 | v_h1(64) | 1]
        v_ext = resid.tile([128, NT * VW], F32R, tag="vext")
        ones32 = consts.tile([128, NT], F32)
        nc.vector.memset(ones32[:], 1.0)
        vxr = v_ext.rearrange("p (j w) -> p j w", w=VW)
        nc.vector.tensor_copy(vxr[:, :, 64:65], ones32[:, :, None])
        nc.vector.tensor_copy(vxr[:, :, 129:130], ones32[:, :, None])

        # ---- Phase 1: norm scales, transpose, projections ----------------
        _nsup = int(os.environ.get("KERN_NSUP", NSUP))
        for ts in range(_nsup):
            xts = []
            for tt in range(4):
                t0 = (ts * 4 + tt) * 128
                x_t = xpool.tile([128, D], F32, tag="x")
                nc.sync.dma_start(x_t[:], x[t0:t0 + 128, :])
                scr = scpool.tile([128, D], F32, tag="scr")
                ssq = spool.tile([128, 1], F32, tag="s")
                nc.scalar.activation(scr[:], x_t[:], AF.Square, accum_out=ssq[:])
                nrm = spool.tile([128, 1], F32, tag="s")
                # sqrt(ssq/D) = ||x||/sqrt(D); clamp matches ref max(||x||,1e-12)
                nc.scalar.activation(nrm[:], ssq[:], AF.Sqrt, scale=1.0 / D)
                nc.vector.tensor_scalar_max(nrm[:], nrm[:], 1e-12 / (D ** 0.5))
                s_t = spool.tile([128, 1], F32, tag="s")
                nc.vector.reciprocal(s_t[:], nrm[:])
                # x <- s * x (row scale)
                nc.vector.tensor_scalar(
                    out=x_t[:], in0=x_t[:], scalar1=s_t[:], scalar2=None,
                    op0=ALU.mult,
                )
                xts.append(x_t)

            xt = xtpool.tile([128, DC, 512], F32R, tag="xt")
            for c in range(DC):
                pxt = psB.tile([128, 512], F32, tag="psB")
                for tt in range(4):
                    nc.tensor.matmul(
                        pxt[:, tt * 128:(tt + 1) * 128],
                        xts[tt][:, c * 128:(c + 1) * 128],
                        ident[:],
                        is_transpose=True, start=(tt == 0), stop=(tt == 3),
                    )
                nc.vector.tensor_copy(xt[:, c, :], pxt[:])

            for name in ("q", "k", "v"):
                pp = psA.tile([128, 512], F32, tag="psA")
                wt = w_sb[name]
                for c in range(DC):
                    nc.tensor.matmul(
                        pp[:], wt[:, c, :], xt[:, c, :],
                        start=(c == 0), stop=(c == DC - 1),
                    )
                if name == "q":
                    nc.vector.tensor_copy(qT[:, ts * 512:(ts + 1) * 512], pp[:])
                elif name == "k":
                    nc.vector.tensor_copy(kT[:, ts * 512:(ts + 1) * 512], pp[:])
                else:
                    vt = vtpool.tile([128, 512], F32, tag="vt")
                    nc.vector.tensor_copy(vt[:], pp[:])
                    for tt in range(4):
                        jj = ts * 4 + tt
                        pv = psC.tile([128, 128], F32, tag="psC")
                        nc.tensor.matmul(
                            pv[:], vt[:, tt * 128:(tt + 1) * 128],
                            ident[:], is_transpose=True,
                        )
                        nc.vector.tensor_copy(
                            v_ext[:, jj * VW:jj * VW + 64], pv[:, 0:64])
                        nc.vector.tensor_copy(
                            v_ext[:, jj * VW + 65:jj * VW + 129], pv[:, 64:128])

        # ---- Phase 2: attention ------------------------------------------
        _phase = os.environ.get("KERN_PHASE", "full")
        _norow = _phase == "att_norowtile"
        _noexp = _phase == "att_noexp"
        _nomask = _phase == "att_nomask"
        _nooutT = _phase == "att_nooutT"
        if _norow:
            qTh = [resid.tile([64, N], F32R, tag=f"qT{h}", name=f"qTh{h}")
                   for h in range(2)]
            kTh = [resid.tile([64, N], F32R, tag=f"kT{h}", name=f"kTh{h}")
                   for h in range(2)]
            for h in range(2):
                nc.vector.tensor_copy(qTh[h][:], qT[h * 64:(h + 1) * 64, :])
                nc.vector.tensor_copy(kTh[h][:], kT[h * 64:(h + 1) * 64, :])
        for ib in range(0 if _phase == "noattn" else _nsup):
            njb = 4 * ib + 4
            i0 = ib * 512
            avs = [psB.tile([65, 512], F32, tag="psB", name=f"av{_h}")
                   for _h in range(2)]
            for jg in range(0, njb, 2):
                for head in range(2):
                    hb = head * 64
                    for jb in (jg, jg + 1):
                        if jb >= njb:
                            break
                        m = jb - 4 * ib
                        off = 128 * m if m >= 0 else 0
                        pss = psA.tile([128, 512], F32, tag="psA")
                        if _norow:
                            nc.tensor.matmul(
                                pss[:, off:512],
                                kTh[head][:, jb * 128:(jb + 1) * 128],
                                qTh[head][:, i0 + off:i0 + 512],
                            )
                        else:
                            nc.tensor.matmul(
                                pss[:, off:512],
                                kT[hb:hb + 64, jb * 128:(jb + 1) * 128],
                                qT[hb:hb + 64, i0 + off:i0 + 512],
                            )
                        ex = expool.tile([128, 512], F32R, tag="ex")
                        if _noexp:
                            nc.vector.tensor_copy(ex[:, off:512], pss[:, off:512])
                        else:
                            nc.scalar.activation(
                                ex[:, off:512], pss[:, off:512], AF.Exp)
                        if m >= 0 and not _nomask:
                            # zero where key j0+p > query i0+off+f  (diag strip)
                            nc.gpsimd.affine_select(
                                out=ex[:, off:off + 128], in_=ex[:, off:off + 128],
                                compare_op=ALU.is_ge, fill=0.0, base=0,
                                channel_multiplier=-1, pattern=[[1, 128]],
                            )
                        nc.tensor.matmul(
                            avs[head][:, off:512],
                            v_ext[:, jb * VW + head * 65:jb * VW + head * 65 + 65],
                            ex[:, off:512],
                            start=(jb == 0), stop=(jb == njb - 1),
                        )
            for head in range(2):
                o_sb = opool.tile([65, 512], F32, tag="o")
                nc.vector.tensor_copy(o_sb[:], avs[head][:])
                if _nooutT:
                    nc.sync.dma_start(
                        out[head, i0:i0 + 512, :],
                        o_sb[0:64, :].rearrange("d t -> t d"))
                    continue
                pst = psC.tile([128, 260], F32, tag="psC")
                for k4 in range(4):
                    nc.tensor.matmul(
                        pst[:, k4 * 65:(k4 + 1) * 65],
                        o_sb[:, k4 * 128:(k4 + 1) * 128],
                        ident[0:65, 0:65],
                        is_transpose=True, start=(k4 == 0), stop=(k4 == 3),
                    )
                for k4 in range(4):
                    rd = spool.tile([128, 1], F32, tag="s")
                    nc.vector.reciprocal(rd[:], pst[:, k4 * 65 + 64:k4 * 65 + 65])
                    fin = finpool.tile([128, DH], F32, tag="fin")
                    nc.vector.tensor_scalar(
                        out=fin[:], in0=pst[:, k4 * 65:k4 * 65 + 64],
                        scalar1=rd[:], scalar2=None, op0=ALU.mult,
                    )
                    r0 = i0 + k4 * 128
                    nc.sync.dma_start(out[head, r0:r0 + 128, :], fin[:])
        if _phase != "full":
            zf = finpool.tile([128, DH], F32, tag="fin")
            nc.vector.memset(zf[:], 0.0)
            for head in range(2):
                for tt2 in range(NT):
                    nc.sync.dma_start(out[head, tt2 * 128:(tt2 + 1) * 128, :], zf[:])

    nc.compile()
    return nc


_NC = None


def _get_program():
    global _NC
    if _NC is None:
        _NC = build_program()
    return _NC


def make_in_maps(x, gamma, w_qkv):
    x = np.ascontiguousarray(np.asarray(x, dtype=np.float32))
    gamma = np.ascontiguousarray(np.asarray(gamma, dtype=np.float32))
    w_qkv = np.asarray(w_qkv, dtype=np.float32)
    in_maps = []
    for c in range(8):
        b = c // 4
        h0 = 2 * (c % 4)
        in_maps.append({
            "x": x[b],
            "wq": np.ascontiguousarray(w_qkv[:, h0 * 64:(h0 + 2) * 64]),
            "wk": np.ascontiguousarray(w_qkv[:, 512 + h0 * 64:512 + (h0 + 2) * 64]),
            "wv": np.ascontiguousarray(w_qkv[:, 1024 + h0 * 64:1024 + (h0 + 2) * 64]),
            "gamma": gamma,
        })
    return in_maps


def gather_out(results):
    out = np.empty((B, HEADS, N, DH), dtype=np.float32)
    for c in range(8):
        b = c // 4
        h0 = 2 * (c % 4)
        out[b, h0:h0 + 2] = results[c]["out"]
    return out


def kernel(x, gamma, w_qkv, _trace=False):
    from concourse.bass_utils import run_bass_kernel_spmd

    nc = _get_program()
    in_maps = make_in_maps(x, gamma, w_qkv)
    res = run_bass_kernel_spmd(nc, in_maps, list(range(8)), trace=_trace)
    out = gather_out(res.results)
    if _trace:
        return out, res
    return out



# revision 8
# speedup vs baseline: 1.0419x; 1.0419x over previous
"""Causal attention (RMSNorm + QKV proj + causal softmax attention) on 8 TRN2
NeuronCores.

Math (per reference):
    xn   = x / max(||x_row||, 1e-12) * sqrt(D) * gamma
    qkv  = xn @ w_qkv            -> q,k,v heads of dim 64
    q   *= D**-0.5
    out[b,h] = softmax_causal(q k^T) v

Sharding: 16 (batch, head) pairs over 8 cores -> core c gets batch c//4 and
heads {2*(c%4), 2*(c%4)+1}. Each core runs the same single-core Bass program
(SPMD) on its shard; gamma/weight slices are replicated per core.

v2 design notes (vs v1):
  - All matmul operands bf16 (PSUM stays f32): enables FWL weight loads and
    1 cyc/col transposes; tolerance is 2e-2 so bf16 rounding is fine.
  - ScalarE reading PSUM stalls concurrent PE matmuls ~2.4x (measured);
    DVE reading PSUM does not. So sim results are evacuated PSUM->SBUF by
    DVE (fused with the causal mask add), and Exp runs SBUF->SBUF on
    ScalarE. A knob routes every Kth non-diagonal pair directly through
    ScalarE-from-PSUM to rebalance DVE vs PE.
  - Fused per-superblock loop: norm (GpSimd ssq), scale, transposes and
    projections of superblock ts+1 overlap attention of ts across engines.
  - Sim matmuls are always full 512 wide; causal masking is done by adding
    a precomputed -30000 mask tile during the DVE evacuation, then exp -> 0.
"""

import os
import numpy as np
from contextlib import ExitStack

import concourse.bass as bass
import concourse.tile as tile
from concourse import bacc, mybir
from concourse.masks import make_identity

F32 = mybir.dt.float32
BF16 = mybir.dt.bfloat16
AF = mybir.ActivationFunctionType
ALU = mybir.AluOpType

B, N, D = 2, 4096, 1024
HEADS, DH = 8, 64
NT = N // 128      # 32 token tiles
NSUP = N // 512    # 8 token superblocks
DC = D // 128      # 8 contraction chunks
VW = 2 * (DH + 2)  # 132: [v_h0 | 1 | pad | v_h1 | 1 | pad] per key tile (4B-aligned bf16)
MASKV = -30000.0


def _env_int(name, default):
    return int(os.environ.get(name, default))


def build_program():
    nc = bacc.Bacc("TRN2", target_bir_lowering=False, debug=False)

    # Knobs (read at build time; defaults are the shipped config)
    direct_k = _env_int("KERN_DIRECT_K", 3)   # every Kth non-diag pair: exp direct from PSUM (0=never)
    ssq_gps = _env_int("KERN_SSQ_GPS", 0)     # 1: ssq on GpSimd, else DVE
    scale_gps = _env_int("KERN_SCALE_GPS", 0)  # 1: row-scale on GpSimd, else DVE

    x = nc.dram_tensor("x", [N, D], F32, kind="ExternalInput").ap()
    wq = nc.dram_tensor("wq", [D, 128], F32, kind="ExternalInput").ap()
    wk = nc.dram_tensor("wk", [D, 128], F32, kind="ExternalInput").ap()
    wv = nc.dram_tensor("wv", [D, 128], F32, kind="ExternalInput").ap()
    gamma = nc.dram_tensor("gamma", [D], F32, kind="ExternalInput").ap()
    out = nc.dram_tensor("out", [2, N, DH], F32, kind="ExternalOutput").ap()

    with tile.TileContext(nc) as tc, ExitStack() as ctx:
        consts = ctx.enter_context(tc.tile_pool(name="consts", bufs=1))
        wpool = ctx.enter_context(tc.tile_pool(name="wpool", bufs=1))
        xpool = ctx.enter_context(tc.tile_pool(name="xpool", bufs=5))
        xbpool = ctx.enter_context(tc.tile_pool(name="xbpool", bufs=5))
        sqpool = ctx.enter_context(tc.tile_pool(name="sqpool", bufs=2))
        spool = ctx.enter_context(tc.tile_pool(name="spool", bufs=10))
        xtpool = ctx.enter_context(tc.tile_pool(name="xtpool", bufs=2))
        resid = ctx.enter_context(tc.tile_pool(name="resid", bufs=1))
        vtpool = ctx.enter_context(tc.tile_pool(name="vtpool", bufs=2))
        scrpool = ctx.enter_context(tc.tile_pool(name="scrpool", bufs=3))
        expool = ctx.enter_context(tc.tile_pool(name="expool", bufs=4))
        opool = ctx.enter_context(tc.tile_pool(name="opool", bufs=2))
        finpool = ctx.enter_context(tc.tile_pool(name="finpool", bufs=4))
        # PSUM: psA pair tiles 2x2 banks + psB avs 2 + psP shared 2 = 8 banks
        psA = ctx.enter_context(tc.tile_pool(name="psA", bufs=4, space="PSUM"))
        psB = ctx.enter_context(tc.tile_pool(name="psB", bufs=2, space="PSUM"))
        psP = ctx.enter_context(tc.tile_pool(name="psP", bufs=2, space="PSUM"))

        # ---- Phase 0: constants + weights -------------------------------
        ident = consts.tile([128, 128], F32)
        make_identity(nc, ident[:])

        # Causal pair masks: mask[m][p, f] = 0 if f >= p + 128*m else -30000
        # pair tile 0 holds halves m=0,1; pair tile 1 holds m=2,3.
        maskm = []
        for m in range(4):
            mt = consts.tile([128, 512], F32, tag=f"mask{m}", name=f"mask{m}")
            nc.gpsimd.memset(mt[:], 0.0)
            nc.gpsimd.affine_select(
                out=mt[:], in_=mt[:],
                compare_op=ALU.is_ge, fill=MASKV,
                base=-128 * m, channel_multiplier=-1, pattern=[[1, 512]],
            )
            maskm.append(mt)

        w_raw = {}
        w_sb = {}
        for name, w in (("q", wq), ("k", wk), ("v", wv)):
            t = wpool.tile([128, DC, 128], F32, tag=f"wr{name}")
            for c in range(DC):
                nc.sync.dma_start(t[:, c, :], w[c * 128:(c + 1) * 128, :])
            w_raw[name] = t
            w_sb[name] = wpool.tile([128, DC, 128], BF16, tag=f"w{name}", name=f"w_{name}")

        g_sb = wpool.tile([128, DC], F32, tag="g")
        nc.sync.dma_start(g_sb[:], gamma.rearrange("(c p) -> p c", p=128))
        gq_sb = wpool.tile([128, DC], F32, tag="gq")
        nc.scalar.mul(gq_sb[:], g_sb[:], float(D) ** -0.5)

        for name, gt in (("q", gq_sb), ("k", g_sb), ("v", g_sb)):
            for c in range(DC):
                nc.vector.tensor_scalar_mul(
                    w_sb[name][:, c, :], w_raw[name][:, c, :], gt[:, c:c + 1])

        # qT/kT: [128 feat (2 heads x 64), N] resident bf16
        qT = resid.tile([128, N], BF16, tag="qT")
        kT = resid.tile([128, N], BF16, tag="kT")
        # v_ext: per key tile jj: cols [jj*130..) = [v_h0(64) | 1 | v_h1(64) | 1]
        v_ext = resid.tile([128, NT * VW], BF16, tag="vext")
        ones32 = consts.tile([128, NT], BF16)
        nc.vector.memset(ones32[:], 1.0)
        vxr = v_ext.rearrange("p (j w) -> p j w", w=VW)
        nc.vector.tensor_copy(vxr[:, :, 64:65], ones32[:, :, None])
        nc.vector.tensor_copy(vxr[:, :, 130:131], ones32[:, :, None])

        ndpair = 0  # non-diagonal pair counter for the direct-exp knob

        # ---- Fused loop over superblocks --------------------------------
        for ts in range(NSUP):
            # --- norm + scale + cast (4 token tiles) ---
            xbs = []
            for tt in range(4):
                t0 = (ts * 4 + tt) * 128
                x_t = xpool.tile([128, D], F32, tag="x")
                nc.sync.dma_start(x_t[:], x[t0:t0 + 128, :])
                sqscr = sqpool.tile([128, D], F32, tag="sq")
                ssq = spool.tile([128, 1], F32, tag="s")
                if ssq_gps:
                    nc.gpsimd.scalar_tensor_tensor(
                        out=sqscr[:], in0=x_t[:], scalar=1.0, in1=x_t[:],
                        op0=ALU.mult, op1=ALU.mult, accum_out=ssq[:])
                else:
                    nc.vector.scalar_tensor_tensor(
                        out=sqscr[:], in0=x_t[:], scalar=1.0, in1=x_t[:],
                        op0=ALU.mult, op1=ALU.mult, accum_out=ssq[:])
                nrm = spool.tile([128, 1], F32, tag="s")
                # sqrt(ssq/D) = ||x||/sqrt(D); clamp matches ref max(||x||,1e-12)
                nc.scalar.activation(nrm[:], ssq[:], AF.Sqrt, scale=1.0 / D)
                nc.vector.tensor_scalar_max(nrm[:], nrm[:], 1e-12 / (D ** 0.5))
                s_t = spool.tile([128, 1], F32, tag="s")
                nc.vector.reciprocal(s_t[:], nrm[:])
                xb = xbpool.tile([128, D], F32, tag="xb")
                if scale_gps:
                    nc.gpsimd.tensor_scalar(
                        out=xb[:], in0=x_t[:], scalar1=s_t[:], scalar2=None,
                        op0=ALU.mult)
                else:
                    nc.vector.tensor_scalar(
                        out=xb[:], in0=x_t[:], scalar1=s_t[:], scalar2=None,
                        op0=ALU.mult)
                xbs.append(xb)

            # --- transpose to d-major (bf16) ---
            xt = xtpool.tile([128, DC, 512], BF16, tag="xt")
            for c in range(DC):
                pxt = psP.tile([128, 512], F32, tag="ps5", name="pxt")
                for tt in range(4):
                    nc.tensor.matmul(
                        pxt[:, tt * 128:(tt + 1) * 128],
                        xbs[tt][:, c * 128:(c + 1) * 128],
                        ident[:],
                        is_transpose=True, start=(tt == 0), stop=(tt == 3),
                    )
                nc.vector.tensor_copy(xt[:, c, :], pxt[:])

            # --- projections ---
            for name in ("q", "k", "v"):
                pp = psP.tile([128, 512], F32, tag="ps5", name="pp")
                wt = w_sb[name]
                for c in range(DC):
                    nc.tensor.matmul(
                        pp[:], wt[:, c, :], xt[:, c, :],
                        start=(c == 0), stop=(c == DC - 1),
                    )
                if name == "q":
                    nc.vector.tensor_copy(qT[:, ts * 512:(ts + 1) * 512], pp[:])
                elif name == "k":
                    nc.vector.tensor_copy(kT[:, ts * 512:(ts + 1) * 512], pp[:])
                else:
                    vt = vtpool.tile([128, 512], F32, tag="vt")
                    nc.vector.tensor_copy(vt[:], pp[:])
                    for tt in range(4):
                        jj = ts * 4 + tt
                        pv = psP.tile([128, 128], F32, tag="ps5", name="pv")
                        nc.tensor.matmul(
                            pv[:], vt[:, tt * 128:(tt + 1) * 128],
                            ident[:], is_transpose=True,
                        )
                        dst = vxr[:, jj, 0:132].rearrange("p (h w) -> p h w", h=2)
                        nc.vector.tensor_copy(
                            dst[:, :, 0:64],
                            pv.rearrange("p (h w) -> p h w", h=2))

            # --- attention for ib = ts ---
            ib = ts
            njb = 4 * ib + 4
            i0 = ib * 512
            avs = [psB.tile([65, 512], F32, tag="psB", name=f"av{_h}")
                   for _h in range(2)]
            for jg in range(0, njb, 2):
                for head in range(2):
                    hb = head * 64
                    for jb in (jg, jg + 1):
                        m = jb - 4 * ib
                        pss = psA.tile([128, 512], F32, tag="psA")
                        nc.tensor.matmul(
                            pss[:],
                            kT[hb:hb + 64, jb * 128:(jb + 1) * 128],
                            qT[hb:hb + 64, i0:i0 + 512],
                        )
                        ex = expool.tile([128, 512], BF16, tag="ex")
                        if m >= 0:
                            scr = scrpool.tile([128, 512], F32, tag="scr")
                            nc.vector.scalar_tensor_tensor(
                                out=scr[:], in0=pss[:], scalar=1.0,
                                in1=maskm[m][:], op0=ALU.mult, op1=ALU.add)
                            nc.scalar.activation(ex[:], scr[:], AF.Exp)
                        else:
                            ndpair += 1
                            if direct_k and ndpair % direct_k == 0:
                                nc.scalar.activation(ex[:], pss[:], AF.Exp)
                            else:
                                scr = scrpool.tile([128, 512], F32, tag="scr")
                                nc.vector.tensor_copy(scr[:], pss[:])
                                nc.scalar.activation(ex[:], scr[:], AF.Exp)
                        nc.tensor.matmul(
                            avs[head][:],
                            v_ext[:, jb * VW + head * 66:jb * VW + head * 66 + 65],
                            ex[:],
                            start=(jb == 0), stop=(jb == njb - 1),
                        )
            for head in range(2):
                o_sb = opool.tile([65, 512], F32, tag="o")
                nc.vector.tensor_copy(o_sb[:], avs[head][:])
                pst = psP.tile([128, 264], F32, tag="ps5", name="pst")
                for k4 in range(4):
                    nc.tensor.matmul(
                        pst[:, k4 * 66:k4 * 66 + 65],
                        o_sb[:, k4 * 128:(k4 + 1) * 128],
                        ident[0:65, 0:65],
                        is_transpose=True, start=(k4 == 0), stop=(k4 == 3),
                    )
                for k4 in range(4):
                    rd = spool.tile([128, 1], F32, tag="s")
                    nc.vector.reciprocal(rd[:], pst[:, k4 * 66 + 64:k4 * 66 + 65])
                    fin = finpool.tile([128, DH], F32, tag="fin")
                    nc.vector.tensor_scalar(
                        out=fin[:], in0=pst[:, k4 * 66:k4 * 66 + 64],
                        scalar1=rd[:], scalar2=None, op0=ALU.mult,
                    )
                    r0 = i0 + k4 * 128
                    nc.sync.dma_start(out[head, r0:r0 + 128, :], fin[:])

    nc.compile()
    return nc


_NC = None


def _get_program():
    global _NC
    if _NC is None:
        _NC = build_program()
    return _NC


def make_in_maps(x, gamma, w_qkv):
    x = np.ascontiguousarray(np.asarray(x, dtype=np.float32))
    gamma = np.ascontiguousarray(np.asarray(gamma, dtype=np.float32))
    w_qkv = np.asarray(w_qkv, dtype=np.float32)
    in_maps = []
    for c in range(8):
        b = c // 4
        h0 = 2 * (c % 4)
        in_maps.append({
            "x": x[b],
            "wq": np.ascontiguousarray(w_qkv[:, h0 * 64:(h0 + 2) * 64]),
            "wk": np.ascontiguousarray(w_qkv[:, 512 + h0 * 64:512 + (h0 + 2) * 64]),
            "wv": np.ascontiguousarray(w_qkv[:, 1024 + h0 * 64:1024 + (h0 + 2) * 64]),
            "gamma": gamma,
        })
    return in_maps


def gather_out(results):
    out = np.empty((B, HEADS, N, DH), dtype=np.float32)
    for c in range(8):
        b = c // 4
        h0 = 2 * (c % 4)
        out[b, h0:h0 + 2] = results[c]["out"]
    return out


def kernel(x, gamma, w_qkv, _trace=False):
    from concourse.bass_utils import run_bass_kernel_spmd

    nc = _get_program()
    in_maps = make_in_maps(x, gamma, w_qkv)
    res = run_bass_kernel_spmd(nc, in_maps, list(range(8)), trace=_trace)
    out = gather_out(res.results)
    if _trace:
        return out, res
    return out
